# revision 1
# baseline (speedup 1.0000x reference)
"""GAT layer kernel for Trainium2, 8 NeuronCores.

Sharding: 16 (b, h) pairs -> 8 cores. Core k handles batch b = k//2 and the
head pair hp = k%2 (heads 2*hp, 2*hp+1). adj (as an additive fp16 mask, host
pre-transposed) is replicated; each core runs the full N^2 attention for its
two heads, then the pair of cores for one batch AllReduces the partial output
of the head-mixing linear.

Math per (b, h), with softmax over the *i* axis (rows) of e[i, j]:
  h    = x[b] @ W[h]                         [N, F]
  f1_i = h_i . a1,  f2_j = h_j . a2
  v[j, i]  = f1_i + f2_j + M[j, i]           (M = 0 on edge, -150 masked)
  L        = max(v, 0.2*v)                   (= leaky, exp-monotone safe)
  Em[j, i] = exp(L)    ;  s_j = sum_i Em[j, i]   (ACT accum_out, fused)
  g[j, :]  = h[j, :] / s_j
  hpT[f, i] = sum_j g[j, f] * Em[j, i]       (PE, transposed-out layout)
  out = leaky(relu(hp) cat-heads @ Wl.T + bl)
"""

import sys

import numpy as np

sys.path.insert(0, "/opt/trn_rl_repo")

from concourse import bacc, bass, dve_ops, mybir, tile  # noqa: E402
from concourse.bass_utils import run_bass_kernel_spmd  # noqa: E402
from concourse.dve_spec import C0, C1, C2, Spec, Src0, Src1, relu  # noqa: E402

# Fused leaky-relu of a masked outer sum, one DVE pass at 1x:
#   out = leaky(in0 + s0 + in1) = s1*v + imm2*relu(v),  v = in0 + s0 + in1
# (in0 = broadcast f1 row, s0 = per-partition f2, in1 = additive adj mask).
_v = (Src0 + C0) + Src1
LEAKY_MASK_ANT = dve_ops.DveOp(
    "LEAKY_MASK_ANT",
    Spec(
        body=_v * C1 + relu(_v) * C2,
        reference=lambda in0, in1, s0, s1, imm2: (
            lambda v: (v * s1 + np.maximum(v, 0) * imm2).astype(np.float32)
        )(in0.astype(np.float32) + s0 + in1),
    ),
    subdim=False,
    uops_sha={"v3": "61445124be53cf8e", "v4": "fd84e7f03d2c00e0"},
)
if LEAKY_MASK_ANT.name not in dve_ops._SUB_OPCODE_FOR_NAME:
    dve_ops.OPS.append(LEAKY_MASK_ANT)
    dve_ops._SUB_OPCODE_FOR_NAME[LEAKY_MASK_ANT.name] = (
        dve_ops._CUSTOM_DVE_ROW_BASE + len(dve_ops.OPS) - 1)
    dve_ops.CUSTOM_DVE_SPECS[LEAKY_MASK_ANT.name] = LEAKY_MASK_ANT.spec

B, N, C, F, H = 4, 2048, 256, 64, 4
P = 128
NT = N // P  # 16 j-tiles / n-chunks
CT = C // P  # 2 contraction tiles over Cin
IC = 512  # i-chunk (matmul moving free dim / psum bank)
NIC = N // IC  # 4
ALPHA = 0.2
MASKV = 150.0  # additive mask magnitude; exp(0.2 * -150) ~ 1e-13
NCORES = 8

F32 = mybir.dt.float32
F16 = mybir.dt.float16
ADD = mybir.AluOpType.add
MULT = mybir.AluOpType.mult
MAX = mybir.AluOpType.max

_CACHE = {}


def _build_program(host_combine=False):
    nc = bacc.Bacc("TRN2", target_bir_lowering=False, debug=False,
                   num_devices=NCORES)

    xT = nc.dram_tensor("xT", [C, N], F32, kind="ExternalInput")
    madd = nc.dram_tensor("madd", [N, N], F16, kind="ExternalInput")
    w = nc.dram_tensor("w", [2, C, F], F32, kind="ExternalInput")
    a1c = nc.dram_tensor("a1c", [2, F], F32, kind="ExternalInput")
    a2c = nc.dram_tensor("a2c", [2, F], F32, kind="ExternalInput")
    wlT = nc.dram_tensor("wlT", [P, F], F16, kind="ExternalInput")
    blt = nc.dram_tensor("blt", [P, NT * F], F32, kind="ExternalInput")
    out = nc.dram_tensor("out", [N, F], F32, kind="ExternalOutput")

    cc_in = nc.dram_tensor("cc_in", [N, F], F32)
    cc_out = nc.dram_tensor("cc_out", [N, F], F32)

    with tile.TileContext(nc) as tc:
        with (
            tc.tile_pool(name="const", bufs=1) as const,
            tc.tile_pool(name="head", bufs=1) as head,
            tc.tile_pool(name="vm", bufs=3) as vm_pool,
            tc.tile_pool(name="em", bufs=3) as em_pool,
            tc.tile_pool(name="g", bufs=4) as g_pool,
            tc.tile_pool(name="psA", bufs=2, space="PSUM") as psA,
            tc.tile_pool(name="psB", bufs=1, space="PSUM") as psB,
        ):
            # ---- constant loads -------------------------------------------
            xT_sb = const.tile([P, CT, N], F32)
            for ct in range(CT):
                nc.sync.dma_start(xT_sb[:, ct, :], xT[ct * P:(ct + 1) * P, :])
            madd_sb = [const.tile([P, N], F16, tag=f"madd{j}",
                                  name=f"madd_sb{j}")
                       for j in range(NT)]
            for jt in range(NT):
                nc.sync.dma_start(madd_sb[jt][:],
                                  madd[jt * P:(jt + 1) * P, :])
            w_sb = const.tile([P, 2, CT, F], F32)
            for hl in range(2):
                for ct in range(CT):
                    nc.sync.dma_start(w_sb[:, hl, ct, :],
                                      w[hl, ct * P:(ct + 1) * P, :])
            a1_sb = const.tile([F, 2], F32)
            a2_sb = const.tile([F, 2], F32)
            for hl in range(2):
                nc.sync.dma_start(a1_sb[:, hl:hl + 1],
                                  a1c[hl:hl + 1, :].rearrange("a f -> f a"))
                nc.sync.dma_start(a2_sb[:, hl:hl + 1],
                                  a2c[hl:hl + 1, :].rearrange("a f -> f a"))
            wlT_sb = const.tile([P, F], F16)
            nc.sync.dma_start(wlT_sb[:], wlT[:])
            blt_sb = const.tile([P, NT * F], F32)
            nc.sync.dma_start(blt_sb[:], blt[:])
            ones_sb = const.tile([1, P], F32)
            nc.vector.memset(ones_sb[:], 1.0)

            catT_sb = const.tile([P, N], F16)

            for hl in range(2):
                # ---- phase A: projections -------------------------------
                h_sb = head.tile([P, NT, F], F16, tag="h")
                hT_sb = head.tile([F, N], F32, tag="hT")
                f1r_sb = head.tile([1, N], F32, tag="f1r")
                F1B_sb = head.tile([P, N], F16, tag="F1B")
                f2c_sb = head.tile([P, NT], F32, tag="f2c")
                sc_sb = head.tile([P, NT], F32, tag="sc")
                rc_sb = head.tile([P, NT], F32, tag="rc")

                # hT[f, n] = sum_c W[c, f] * xT[c, n]
                for icc in range(NIC):
                    ps = psA.tile([F, IC], F32, tag="psum_a")
                    for ct in range(CT):
                        nc.tensor.matmul(
                            ps[:], w_sb[:, hl, ct, :],
                            xT_sb[:, ct, icc * IC:(icc + 1) * IC],
                            start=(ct == 0), stop=(ct == CT - 1))
                    nc.any.tensor_copy(hT_sb[:, icc * IC:(icc + 1) * IC],
                                       ps[:])
                # h[n, f] = sum_c xT[c, n] * W[c, f]   (fp16 for pass-2 g)
                # 8 n-chunks packed per psum bank -> 2 big copies
                for grp in range(2):
                    ps = psA.tile([P, IC], F32, tag="psum_a")
                    for k in range(8):
                        nt = grp * 8 + k
                        for ct in range(CT):
                            nc.tensor.matmul(
                                ps[:, k * F:(k + 1) * F],
                                xT_sb[:, ct, nt * P:(nt + 1) * P],
                                w_sb[:, hl, ct, :],
                                start=(ct == 0), stop=(ct == CT - 1))
                    nc.any.tensor_copy(
                        h_sb[:, grp * 8:(grp + 1) * 8, :],
                        ps[:].rearrange("p (k f) -> p k f", f=F))
                # f1 row [1, N] = a1 . hT  ;  f2 col per tile = hT.T @ a2
                for icc in range(NIC):
                    ps = psA.tile([1, IC], F32, tag="psum_a")
                    nc.tensor.matmul(ps[:], a1_sb[:, hl:hl + 1],
                                     hT_sb[:, icc * IC:(icc + 1) * IC],
                                     start=True, stop=True)
                    nc.any.tensor_copy(f1r_sb[:, icc * IC:(icc + 1) * IC],
                                       ps[:])
                ps_f2 = psA.tile([P, NT], F32, tag="psum_a")
                for jt in range(NT):
                    nc.tensor.matmul(ps_f2[:, jt:jt + 1],
                                     hT_sb[:, jt * P:(jt + 1) * P],
                                     a2_sb[:, hl:hl + 1],
                                     start=True, stop=True)
                nc.any.tensor_copy(f2c_sb[:], ps_f2[:])
                # F1B = broadcast f1 row across partitions (ones outer-prod)
                for icc in range(NIC):
                    ps = psA.tile([P, IC], F32, tag="psum_a")
                    nc.tensor.matmul(ps[:], ones_sb[:],
                                     f1r_sb[:, icc * IC:(icc + 1) * IC],
                                     start=True, stop=True)
                    nc.any.tensor_copy(F1B_sb[:, icc * IC:(icc + 1) * IC],
                                       ps[:])

                # ---- hot loop: masked exp-leaky attention ---------------
                hpT = psB.tile([P, N], F32, tag="hpT")
                for jt in range(NT):
                    lk = vm_pool.tile([P, N], F16, tag="lk")
                    nc.vector._custom_dve(
                        LEAKY_MASK_ANT, out=lk[:], in0=F1B_sb[:],
                        in1=madd_sb[jt][:], s0=f2c_sb[:, jt:jt + 1],
                        s1=float(ALPHA), imm2=1.0 - ALPHA)
                    em = em_pool.tile([P, N], F16, tag="em")
                    nc.scalar.activation(
                        em[:], lk[:], mybir.ActivationFunctionType.Exp,
                        accum_out=sc_sb[:, jt:jt + 1])
                    nc.vector.reciprocal(rc_sb[:, jt:jt + 1],
                                         sc_sb[:, jt:jt + 1])
                    g = g_pool.tile([P, F], F16, tag="g")
                    nc.vector.tensor_scalar_mul(g[:], h_sb[:, jt, :],
                                                rc_sb[:, jt:jt + 1])
                    for icc in range(NIC):
                        nc.tensor.matmul(
                            hpT[hl * F:(hl + 1) * F,
                                icc * IC:(icc + 1) * IC],
                            g[:], em[:, icc * IC:(icc + 1) * IC],
                            start=(jt == 0), stop=(jt == NT - 1))
                # relu(hp) into the concat-head tile (same partitions)
                nc.scalar.activation(catT_sb[hl * F:(hl + 1) * F, :],
                                     hpT[hl * F:(hl + 1) * F, :],
                                     mybir.ActivationFunctionType.Relu)

            # ---- phase C: head-mixing linear + pair AllReduce -----------
            part_sb = const.tile([P, NT * F], F32)
            for grp in range(2):
                ps = psA.tile([P, IC], F32, tag="psum_a")
                for k in range(8):
                    ncu = grp * 8 + k
                    nc.tensor.matmul(ps[:, k * F:(k + 1) * F],
                                     catT_sb[:, ncu * P:(ncu + 1) * P],
                                     wlT_sb[:], start=True, stop=True)
                nc.any.tensor_copy(
                    part_sb[:, grp * IC:(grp + 1) * IC], ps[:])
            if host_combine:
                nc.sync.dma_start(
                    out.rearrange("(c p) f -> p c f", p=P),
                    part_sb[:].rearrange("p (c f) -> p c f", f=F))
            else:
                cc_in_v = cc_in.rearrange("(c p) f -> p c f", p=P)
                nc.sync.dma_start(cc_in_v, part_sb[:].rearrange(
                    "p (c f) -> p c f", f=F))
                nc.gpsimd.collective_compute(
                    "AllReduce", ADD,
                    replica_groups=[[0, 1], [2, 3], [4, 5], [6, 7]],
                    ins=[cc_in[:]], outs=[cc_out[:]])
                ys_sb = const.tile([P, NT * F], F32)
                nc.sync.dma_start(
                    ys_sb[:].rearrange("p (c f) -> p c f", f=F),
                    cc_out.rearrange("(c p) f -> p c f", p=P))
                yb_sb = const.tile([P, NT * F], F32)
                nc.vector.tensor_tensor(yb_sb[:], ys_sb[:], blt_sb[:],
                                        op=ADD)
                yo_sb = const.tile([P, NT * F], F32)
                nc.vector.scalar_tensor_tensor(
                    yo_sb[:], yb_sb[:], float(ALPHA), yb_sb[:],
                    op0=MULT, op1=MAX)
                nc.sync.dma_start(
                    out.rearrange("(c p) f -> p c f", p=P),
                    yo_sb[:].rearrange("p (c f) -> p c f", f=F))

    nc.compile()
    return nc


def get_program(host_combine=False):
    key = ("nc", host_combine)
    if key not in _CACHE:
        _CACHE[key] = _build_program(host_combine)
    return _CACHE[key]


def make_in_maps(x, adj, W, a1, a2, Wl, bl):
    x = np.asarray(x, dtype=np.float32)
    adj = np.asarray(adj)
    W = np.asarray(W, dtype=np.float32)
    a1 = np.asarray(a1, dtype=np.float32)
    a2 = np.asarray(a2, dtype=np.float32)
    Wl = np.asarray(Wl, dtype=np.float32)
    bl = np.asarray(bl, dtype=np.float32)

    madd = ((MASKV * adj.T.astype(np.float32)) - MASKV).astype(np.float16)
    madd = np.ascontiguousarray(madd)
    WlT = np.ascontiguousarray(Wl.T)  # [H*F, F]
    blt = np.ascontiguousarray(np.tile(bl, (P, NT)))

    in_maps = []
    for k in range(NCORES):
        b, hp = k // 2, k % 2
        hs = slice(2 * hp, 2 * hp + 2)
        in_maps.append({
            "xT": np.ascontiguousarray(x[b].T),
            "madd": madd,
            "w": np.ascontiguousarray(W[hs]),
            "a1c": np.ascontiguousarray(a1[hs]),
            "a2c": np.ascontiguousarray(a2[hs]),
            "wlT": np.ascontiguousarray(
                WlT[hp * P:(hp + 1) * P]).astype(np.float16),
            "blt": blt,
        })
    return in_maps


def kernel(x, adj, W, a1, a2, Wl, bl, _results=None, host_combine=False,
           **run_kwargs):
    nc = get_program(host_combine)
    in_maps = make_in_maps(x, adj, W, a1, a2, Wl, bl)
    res = run_bass_kernel_spmd(nc, in_maps, core_ids=list(range(NCORES)),
                               **run_kwargs)
    if _results is not None:
        _results.append(res)
    out = np.empty((B, N, F), dtype=np.float32)
    if host_combine:
        bl32 = np.asarray(bl, dtype=np.float32)
        for b in range(B):
            y = (res.results[2 * b]["out"] + res.results[2 * b + 1]["out"]
                 + bl32[None, :])
            out[b] = np.maximum(y, ALPHA * y)
    else:
        for b in range(B):
            out[b] = res.results[2 * b]["out"]
    return out



# revision 6
# speedup vs baseline: 1.4589x; 1.4589x over previous
"""GAT layer kernel for Trainium2, 8 NeuronCores — v2.

Sharding: 16 (b, h) pairs -> 8 cores. Core k handles batch b = k//2 and the
head pair hp = k%2 (heads 2*hp, 2*hp+1). adj (as an additive int8 mask, host
pre-transposed) is replicated; each core runs the full N^2 attention for its
two heads, then the pair of cores for one batch AllReduces (fp16) the partial
output of the head-mixing linear.

Math per (b, h), with softmax over the *i* axis (rows) of e[i, j]:
  h    = x[b] @ W[h]                         [N, F]
  f1_i = h_i . a1 = x_i . (W a1),  f2_j = x_j . (W a2)
  v[j, i]  = f1_i + f2_j + M[j, i]           (M = 0 on edge, -120 masked)
  L        = max(v, 0.2*v)                   (= leaky, exp-monotone safe)
  Em[j, i] = exp(L)    ;  s_j = sum_i Em[j, i]   (ACT accum_out, fused)
  g[j, :]  = h[j, :] / s_j
  hpT[f, i] = sum_j g[j, f] * Em[j, i]       (PE, transposed-out layout)
  out = leaky(relu(hp) cat-heads @ Wl.T + bl)

v2 structure:
  - x/W/Wa1/Wa2 in fp16, adj mask in int8 -> input DMA ~5.5MB (was 12.7).
  - Few large DMAs (consts first, madd in 4 groups on the gpsimd queue).
  - hT eliminated: f1 comes from a replicated-Wa1 stationary so the PE
    emits the broadcast F1B tile directly; f2 column comes from Wa2.
  - phase A of head 1 is emitted before hot loop 0 (head pool bufs=2) so
    projections/copies overlap the previous head's hot loop.
  - per-tile g scaling on GpSimd; PSUM->SBUF copies on ScalarE.
  - fp16 AllReduce + fp16 bias/leaky tail + fp16 output (host casts f32).
"""

import sys

import numpy as np

sys.path.insert(0, "/opt/trn_rl_repo")

from concourse import bacc, bass, dve_ops, mybir, tile  # noqa: E402
from concourse.bass_utils import run_bass_kernel_spmd  # noqa: E402

sys.path.insert(0, "/root/problem")
import dve2x  # noqa: E402

# Fused leaky-relu of a masked outer sum: out = max(v, alpha*v),
# v = in0 + s0 + in1 (in0 = broadcast f1 row, s0 = per-partition f2,
# in1 = additive adj mask). Hand-written 2X_1PORT uop program: ~2x DVE
# throughput for fp16 operands (see dve2x.py).
LEAKY2_MASK_ANT = dve2x.register()

B, N, C, F, H = 4, 2048, 256, 64, 4
P = 128
NT = N // P  # 16 j-tiles / n-chunks
CT = C // P  # 2 contraction tiles over Cin
IC = 512  # i-chunk (matmul moving free dim / psum bank)
NIC = N // IC  # 4
ALPHA = 0.2
MASKV = 120.0  # additive int8 mask magnitude; exp(0.2 * -120) ~ 4e-11
NCORES = 8

F32 = mybir.dt.float32
F16 = mybir.dt.float16
I8 = mybir.dt.int8
ADD = mybir.AluOpType.add
MULT = mybir.AluOpType.mult
MAX = mybir.AluOpType.max

_CACHE = {}


def _build_program():
    nc = bacc.Bacc("TRN2", target_bir_lowering=False, debug=False,
                   num_devices=NCORES)

    w16 = nc.dram_tensor("w16", [P, 2, CT, F], F16, kind="ExternalInput")
    wa1rep = nc.dram_tensor("wa1rep", [P, 2, CT, P], F16,
                            kind="ExternalInput")
    wa2c = nc.dram_tensor("wa2c", [P, 2, CT, 1], F16, kind="ExternalInput")
    wlT = nc.dram_tensor("wlT", [P, F], F16, kind="ExternalInput")
    blt = nc.dram_tensor("blt", [P, NT * F], F16, kind="ExternalInput")
    xt16 = nc.dram_tensor("xt16", [P, CT, N], F16, kind="ExternalInput")
    madd16 = nc.dram_tensor("madd16", [N, N], F16, kind="ExternalInput")
    out16 = nc.dram_tensor("out16", [N, F], F16, kind="ExternalOutput")

    cc_in = nc.dram_tensor("cc_in", [N, F], F16)
    cc_out = nc.dram_tensor("cc_out", [N, F], F16)

    with tile.TileContext(nc) as tc:
        with (
            tc.tile_pool(name="const", bufs=1) as const,
            tc.tile_pool(name="head", bufs=2) as head,
            tc.tile_pool(name="vm", bufs=3) as vm_pool,
            tc.tile_pool(name="em", bufs=3) as em_pool,
            tc.tile_pool(name="g", bufs=4) as g_pool,
            tc.tile_pool(name="psA", bufs=2, space="PSUM") as psA,
            tc.tile_pool(name="psF", bufs=2, space="PSUM") as psF,
            tc.tile_pool(name="psB", bufs=1, space="PSUM") as psB,
        ):
            # ---- constant loads (small consts first, then x, then mask) --
            w_sb = const.tile([P, 2, CT, F], F16)
            nc.sync.dma_start(w_sb[:], w16[:])
            wa1_sb = const.tile([P, 2, CT, P], F16)
            nc.sync.dma_start(wa1_sb[:], wa1rep[:])
            wa2_sb = const.tile([P, 2, CT, 1], F16)
            nc.sync.dma_start(wa2_sb[:], wa2c[:])
            wlT_sb = const.tile([P, F], F16)
            nc.sync.dma_start(wlT_sb[:], wlT[:])
            blt_sb = const.tile([P, NT * F], F16)
            nc.sync.dma_start(blt_sb[:], blt[:])
            xT_sb = const.tile([P, CT, N], F16)
            nc.sync.dma_start(xT_sb[:], xt16[:])
            # mask: 4 DMAs of 4 j-tiles each, issued from the gpsimd queue so
            # the sync queue isn't serialized behind them.
            madd_sb = const.tile([P, NT, N], F16)
            for grp in range(8):
                nc.gpsimd.dma_start(
                    madd_sb[:, 2 * grp:2 * grp + 2, :],
                    madd16[grp * 2 * P:(grp + 1) * 2 * P, :].rearrange(
                        "(t p) i -> p t i", p=P))

            catT_sb = const.tile([P, N], F16)
            hpT = psB.tile([P, N], F32)

            # ---- phase A for both heads (emitted up front; head pool
            # bufs=2 lets head 1's tiles coexist with head 0's) ----------
            heads = []
            for hl in range(2):
                F1B_sb = head.tile([P, N], F16, tag="f1b",
                                   name=f"F1B_sb{hl}")
                f2c_sb = head.tile([P, NT], F32, tag="f2c",
                                   name=f"f2c_sb{hl}")
                h_sb = head.tile([P, NT, F], F16, tag="h", name=f"h_sb{hl}")
                sc_sb = head.tile([P, NT], F32, tag="sc", name=f"sc_sb{hl}")
                rc_sb = head.tile([P, NT], F32, tag="rc", name=f"rc_sb{hl}")
                heads.append((F1B_sb, f2c_sb, h_sb, sc_sb, rc_sb))

                # F1B[j, i] = f1_i via replicated-Wa1 stationary
                for icc in range(NIC):
                    ps = psA.tile([P, IC], F32, tag="psum_a")
                    for ct in range(CT):
                        nc.tensor.matmul(
                            ps[:], wa1_sb[:, hl, ct, :],
                            xT_sb[:, ct, icc * IC:(icc + 1) * IC],
                            start=(ct == 0), stop=(ct == CT - 1))
                    nc.scalar.copy(F1B_sb[:, icc * IC:(icc + 1) * IC],
                                   ps[:])
                # f2 column per j-tile: f2c[p, nt] = x_row(nt*P+p) . Wa2
                psf = psF.tile([P, NT], F32, tag="psum_f")
                for nt in range(NT):
                    for ct in range(CT):
                        nc.tensor.matmul(
                            psf[:, nt:nt + 1],
                            xT_sb[:, ct, nt * P:(nt + 1) * P],
                            wa2_sb[:, hl, ct, :],
                            start=(ct == 0), stop=(ct == CT - 1))
                nc.vector.tensor_copy(f2c_sb[:], psf[:])
                # h[n, f] (fp16, for the g scaling) — 8 n-chunks per bank
                for grp in range(2):
                    ps = psA.tile([P, IC], F32, tag="psum_a")
                    for k in range(8):
                        nt = grp * 8 + k
                        for ct in range(CT):
                            nc.tensor.matmul(
                                ps[:, k * F:(k + 1) * F],
                                xT_sb[:, ct, nt * P:(nt + 1) * P],
                                w_sb[:, hl, ct, :],
                                start=(ct == 0), stop=(ct == CT - 1))
                    nc.scalar.copy(
                        h_sb[:, grp * 8:(grp + 1) * 8, :],
                        ps[:].rearrange("p (k f) -> p k f", f=F))

            # ---- hot loops: masked exp-leaky attention ------------------
            for hl in range(2):
                F1B_sb, f2c_sb, h_sb, sc_sb, rc_sb = heads[hl]
                for jt in range(NT):
                    lk = vm_pool.tile([P, N], F16, tag="lk")
                    bi = nc.vector._custom_dve(
                        LEAKY2_MASK_ANT, out=lk[:], in0=F1B_sb[:],
                        in1=madd_sb[:, jt, :], s0=f2c_sb[:, jt:jt + 1],
                        s1=float(ALPHA))
                    bi.ins.perf_max = 1
                    em = em_pool.tile([P, N], F16, tag="em")
                    nc.scalar.activation(
                        em[:], lk[:], mybir.ActivationFunctionType.Exp,
                        accum_out=sc_sb[:, jt:jt + 1])
                    nc.vector.reciprocal(rc_sb[:, jt:jt + 1],
                                         sc_sb[:, jt:jt + 1])
                    g = g_pool.tile([P, F], F16, tag="g")
                    nc.gpsimd.tensor_scalar_mul(g[:], h_sb[:, jt, :],
                                                rc_sb[:, jt:jt + 1])
                    for icc in range(NIC):
                        nc.tensor.matmul(
                            hpT[hl * F:(hl + 1) * F,
                                icc * IC:(icc + 1) * IC],
                            g[:], em[:, icc * IC:(icc + 1) * IC],
                            start=(jt == 0), stop=(jt == NT - 1))
                # relu(hp) into the concat-head tile (same partitions)
                nc.vector.tensor_scalar_max(catT_sb[hl * F:(hl + 1) * F, :],
                                            hpT[hl * F:(hl + 1) * F, :],
                                            0.0)

            # ---- head-mixing linear + pair AllReduce (fp16) -------------
            part_sb = const.tile([P, NT, F], F16)
            for grp in range(2):
                ps = psA.tile([P, IC], F32, tag="psum_a")
                for k in range(8):
                    ncu = grp * 8 + k
                    nc.tensor.matmul(ps[:, k * F:(k + 1) * F],
                                     catT_sb[:, ncu * P:(ncu + 1) * P],
                                     wlT_sb[:], start=True, stop=True)
                nc.scalar.copy(
                    part_sb[:, grp * 8:(grp + 1) * 8, :],
                    ps[:].rearrange("p (k f) -> p k f", f=F))
            cc_in_v = cc_in.rearrange("(c p) f -> p c f", p=P)
            nc.sync.dma_start(cc_in_v, part_sb[:])
            nc.gpsimd.collective_compute(
                "AllReduce", ADD,
                replica_groups=[[0, 1], [2, 3], [4, 5], [6, 7]],
                ins=[cc_in[:]], outs=[cc_out[:]])
            ys_sb = const.tile([P, NT, F], F16)
            nc.sync.dma_start(
                ys_sb[:], cc_out.rearrange("(c p) f -> p c f", p=P))
            yb_sb = const.tile([P, NT * F], F16)
            nc.vector.tensor_tensor(
                yb_sb[:], ys_sb[:].rearrange("p c f -> p (c f)"),
                blt_sb[:], op=ADD)
            yo_sb = const.tile([P, NT * F], F16)
            nc.vector.scalar_tensor_tensor(
                yo_sb[:], yb_sb[:], float(ALPHA), yb_sb[:],
                op0=MULT, op1=MAX)
            nc.sync.dma_start(
                out16.rearrange("(c p) f -> p c f", p=P),
                yo_sb[:].rearrange("p (c f) -> p c f", f=F))

    nc.compile()
    return nc


def get_program():
    if "nc" not in _CACHE:
        _CACHE["nc"] = _build_program()
    return _CACHE["nc"]


def make_in_maps(x, adj, W, a1, a2, Wl, bl):
    x = np.asarray(x, dtype=np.float32)
    adj = np.asarray(adj)
    W = np.asarray(W, dtype=np.float32)
    a1 = np.asarray(a1, dtype=np.float32)
    a2 = np.asarray(a2, dtype=np.float32)
    Wl = np.asarray(Wl, dtype=np.float32)
    bl = np.asarray(bl, dtype=np.float32)

    madd = ((MASKV * adj.T.astype(np.float32)) - MASKV).astype(np.float16)
    madd = np.ascontiguousarray(madd)
    WlT = np.ascontiguousarray(Wl.T).astype(np.float16)  # [H*F, F]
    blt = np.ascontiguousarray(np.tile(bl, (P, NT))).astype(np.float16)
    wa1 = np.einsum("hcf,hf->hc", W, a1)  # [H, C]
    wa2 = np.einsum("hcf,hf->hc", W, a2)

    in_maps = []
    for k in range(NCORES):
        b, hp = k // 2, k % 2
        hs = slice(2 * hp, 2 * hp + 2)
        # [P, CT, N] fp16 x^T with c = ct*P + p
        xt = np.ascontiguousarray(
            x[b].T.reshape(CT, P, N).transpose(1, 0, 2)).astype(np.float16)
        # [P, 2, CT, F] fp16 W with c = ct*P + p
        w_r = np.ascontiguousarray(
            W[hs].reshape(2, CT, P, F).transpose(2, 0, 1, 3)
        ).astype(np.float16)
        # [P, 2, CT, P] replicated Wa1 stationary: lhsT[c, m] = wa1[c]
        wa1_r = np.ascontiguousarray(np.broadcast_to(
            wa1[hs].reshape(2, CT, P, 1).transpose(2, 0, 1, 3),
            (P, 2, CT, P))).astype(np.float16)
        wa2_r = np.ascontiguousarray(
            wa2[hs].reshape(2, CT, P, 1).transpose(2, 0, 1, 3)
        ).astype(np.float16)
        in_maps.append({
            "xt16": xt,
            "madd16": madd,
            "w16": w_r,
            "wa1rep": wa1_r,
            "wa2c": wa2_r,
            "wlT": np.ascontiguousarray(WlT[hp * P:(hp + 1) * P]),
            "blt": blt,
        })
    return in_maps


def kernel(x, adj, W, a1, a2, Wl, bl, _results=None, **run_kwargs):
    nc = get_program()
    in_maps = make_in_maps(x, adj, W, a1, a2, Wl, bl)
    res = run_bass_kernel_spmd(nc, in_maps, core_ids=list(range(NCORES)),
                               **run_kwargs)
    if _results is not None:
        _results.append(res)
    out = np.empty((B, N, F), dtype=np.float32)
    for b in range(B):
        out[b] = res.results[2 * b]["out16"].astype(np.float32)
    return out
